# revision 18
# baseline (speedup 1.0000x reference)
"""Factored (column) attention kernel for Trainium2, 8 NeuronCores.

Reference computation (B=4, S=4096, D=1024, BLOCK_LEN=128, NB=32):
    qkv = x @ Wqkv + bqkv ; split q,k,v
    'column' attention: each (batch, within-block position bl) row attends
    causally over the NB=32 block indices -> 512 independent length-32
    single-head attentions with head dim 1024.
    out = attn @ Wout + bout

Algebraic folding (the big win vs the first-pass kernel):
  - scores = q k^T = (x Wq + bq)(x Wk + bk)^T; bk cancels in softmax, so
    scores == x A x^T + (bq Wk^T) x^T with A = Wq Wk^T folded host-side.
    One projection q' = x A + bA replaces BOTH q and k projections, and
    the score matmul's k operand is just x^T (already resident in SBUF).
  - out = p v Wout + bout = p (x (Wv Wout)) + (bv Wout + bout) since
    softmax rows sum to 1.  Wf = Wv Wout folded host-side; the entire
    out-projection disappears and p @ v' IS the final output.
  Device matmul work drops from 4 D x D projections + attn to 2
  projections + attn (18.3 -> 9.7 GFLOP per core).

Sharding: data-parallel over the 512 independent (b, bl) attention rows,
64 rows (2048 tokens) per core.  All inputs are re-laid-out host-side so
that on-device matmuls are layout-natural and every DMA is one large
contiguous descriptor (super-tiles: x^T per block is a single [128,4096]
tile, A and Wf single [128,8192] tiles; A additionally ordered so its
first half serves all q' j<4 lhsT slices -> compute starts after 1 MiB):
  - q' is produced in transposed layout [D, tok] (lhsT = A chunk)
  - scores for a 4-group q-pack computed as [K=128,M=128,N=128] matmuls
    per d-chunk: lhsT = q'^T chunk, rhs = x^T chunk (cross-group
    products masked away in softmax)
  - softmax batched on [128,128] tiles; exp+rowsum fused via accum_out;
    normalized p transposed per 32x32 block by one DVE stream-transpose
  - v' = x @ Wf in natural [tok, D] layout (lhsT = x^T chunk)
  - p@v' in natural layout: lhsT = p^T, rhs = v' -> [128 tok, 512 d]
    psum at full N=512; block-diagonal zeros in p^T mask cross-group
    terms exactly; bias (free-axis) added from a replicated tile during
    the PSUM evict; each q-pack's [128,1024] output is one 256KB DMA
Numerics: all matmul operands fp16 (fp32 PSUM accumulation); host-
simulated end-to-end rms error vs the fp32 reference is ~5.5e-4.
"""

import numpy as np

import concourse.bacc as bacc
import concourse.mybir as mybir
import concourse.tile as tile
from concourse.bass_utils import run_bass_kernel_spmd

N_CORES = 8
B, S, D = 4, 4096, 1024
BL = 128          # BLOCK_LEN (within-block positions)
NB = S // BL      # 32 block indices = attention sequence length
NGROUP = B * BL   # 512 independent attention rows
GPC = NGROUP // N_CORES   # 64 groups per core
TOK = GPC * NB    # 2048 tokens per core
BLK = 512         # tokens per fused block (16 groups, 4 q-packs)
NBLK = TOK // BLK  # 4
QP = BLK // 128   # q-packs per block
DC = D // 128     # 8 d-chunks
SCALE = 1.0 / np.sqrt(D)
NEG = -1.0e30

F32 = mybir.dt.float32
F16 = mybir.dt.float16

_PROGRAM = None


def _get_program():
    global _PROGRAM
    if _PROGRAM is None:
        _PROGRAM = _build_program()
    return _PROGRAM


def _build_program():
    nc = bacc.Bacc("TRN2", target_bir_lowering=False, debug=False,
                   num_devices=N_CORES)
    # super-tile layouts (see run() for the host-side packing):
    #   xt[128b + p, 512c + t] = x^T[128c + p, 512b + t]
    #   wa[p, 4096h + 512c + 128j4 + e] = A[128c + p, 128(4h + j4) + e]
    #   wf[p, 1024c + f] = Wf[128c + p, f]
    xt = nc.dram_tensor("xt", [NBLK * 128, DC * BLK], F16,
                        kind="ExternalInput").ap()
    wa = nc.dram_tensor("wa", [128, D * DC], F16, kind="ExternalInput").ap()
    wf = nc.dram_tensor("wf", [128, D * DC], F16, kind="ExternalInput").ap()
    ba = nc.dram_tensor("ba", [D], F32, kind="ExternalInput").ap()
    bo = nc.dram_tensor("bo", [128, D], F32, kind="ExternalInput").ap()
    mask = nc.dram_tensor("mask", [128, 128], F32,
                          kind="ExternalInput").ap()
    ot = nc.dram_tensor("ot", [TOK, D], F16, kind="ExternalOutput").ap()

    with tile.TileContext(nc) as tc:
        with (
            tc.tile_pool(name="wa", bufs=1) as wa_pool,
            tc.tile_pool(name="wf", bufs=1) as wf_pool,
            tc.tile_pool(name="const", bufs=1) as const,
            tc.tile_pool(name="xt", bufs=4) as xt_pool,
            tc.tile_pool(name="qk", bufs=1) as qk_pool,
            tc.tile_pool(name="v", bufs=5) as v_pool,
            tc.tile_pool(name="sm", bufs=6) as sm_pool,
            tc.tile_pool(name="smh", bufs=8) as smh_pool,
            tc.tile_pool(name="small", bufs=8) as small_pool,
            tc.tile_pool(name="out", bufs=4) as out_pool,
            tc.tile_pool(name="psA", bufs=4, space="PSUM") as psA,
            tc.tile_pool(name="psB", bufs=4, space="PSUM") as psB,
        ):
            # warm-up matmuls on a zeroed tile: keep the PE busy (and the
            # HAM clock-gate warm) while the critical first DMAs land
            wu = const.tile([128, 512], F16, tag="warm")
            nc.vector.memset(wu[:], 0.0)
            wu_ps = psB.tile([128, 512], F32, tag="psB", name="wu_ps")
            # sized so the PE never idles before the critical first-block
            # DMAs land (~20us): an idle gap would reset the PE clock
            # ramp and the next ~3us of matmuls would run at half speed
            for _ in range(40):
                nc.tensor.matmul(wu_ps[:], lhsT=wu[:, 0:128], rhs=wu[:],
                                 start=True, stop=True)
            # first-block critical loads split across the HWDGE (sync)
            # and SWDGE (gpsimd) queue sets; each large tile split into
            # a few descriptors so several DMA engines stream in parallel
            # critical first-block loads split across the HWDGE (sync)
            # and SWDGE (gpsimd) queue sets; wa's first half (all q' j<4
            # lhsT slices) leads so compute can start early
            wa_sb = wa_pool.tile([128, D * DC], F16, tag="wa", name="wa")
            for k in range(2):
                nc.sync.dma_start(wa_sb[:, 2048 * k:2048 * (k + 1)],
                                  wa[:, 2048 * k:2048 * (k + 1)])
            xt_sb_all = []
            for b in range(NBLK):
                t = xt_pool.tile([128, DC * BLK], F16, tag="xt",
                                 name=f"xt{b}")
                xt_sb_all.append(t)
            for k in range(4):
                nc.gpsimd.dma_start(xt_sb_all[0][:, 1024 * k:1024 * (k + 1)],
                                    xt[0:128, 1024 * k:1024 * (k + 1)])
            for k in range(2, 4):  # wa second half (q' j>=4 lhsT slices)
                nc.sync.dma_start(wa_sb[:, 2048 * k:2048 * (k + 1)],
                                  wa[:, 2048 * k:2048 * (k + 1)])
            ba_sb = const.tile([128, DC], F32, tag="ba")
            nc.gpsimd.dma_start(ba_sb[:], ba.rearrange("(c p) -> p c", p=128))
            mask_sb = const.tile([128, 128], F32, tag="mask")
            nc.gpsimd.dma_start(mask_sb[:], mask[:])
            wf_sb = wf_pool.tile([128, D * DC], F16, tag="wf", name="wf")
            for k in range(4):
                nc.sync.dma_start(wf_sb[:, 2048 * k:2048 * (k + 1)],
                                  wf[:, 2048 * k:2048 * (k + 1)])
            bo_sb = const.tile([128, D], F32, tag="bo")
            nc.gpsimd.dma_start(bo_sb[:], bo[:])
            for b in range(1, NBLK):
                eng = nc.gpsimd if b == 1 else nc.sync
                for k in range(2):
                    eng.dma_start(
                        xt_sb_all[b][:, 2048 * k:2048 * (k + 1)],
                        xt[128 * b:128 * (b + 1), 2048 * k:2048 * (k + 1)])

            def wa_s(c, j):
                o = 4096 * (j // 4) + 512 * c + 128 * (j % 4)
                return wa_sb[:, o:o + 128]

            pending = []  # previous block's last p@v packs, emitted
            # between this block's q' groups so their v'-evict latency
            # hides behind big matmuls instead of stalling the PE

            for b in range(NBLK):
                xt_sb = xt_sb_all[b]

                # --- q'^T projection: psum [dout-chunk 128, BLK tok]
                qp_sb = [None] * DC
                for j in range(DC):
                    ps = psA.tile([128, BLK], F32, tag="psA")
                    for c in range(DC):
                        nc.tensor.matmul(
                            ps[:],
                            lhsT=wa_s(c, j),
                            rhs=xt_sb[:, 512 * c:512 * (c + 1)],
                            start=(c == 0), stop=(c == DC - 1),
                        )
                    q = qk_pool.tile([128, BLK], F16, tag=f"qk{j}")
                    nc.scalar.add(q[:], ps[:], ba_sb[:, j:j + 1])
                    qp_sb[j] = q
                    if pending and j in (1, 3):
                        pending.pop(0)()

                # --- scores + softmax per 4-group q-pack
                pt_sb = []
                for qp in range(QP):
                    ps = psB.tile([128, 128], F32, tag="psB")
                    for c in range(DC):
                        o = 512 * c + 128 * qp
                        nc.tensor.matmul(
                            ps[:],
                            lhsT=qp_sb[c][:, 128 * qp:128 * (qp + 1)],
                            rhs=xt_sb[:, o:o + 128],
                            start=(c == 0), stop=(c == DC - 1),
                        )
                    tm = sm_pool.tile([128, 128], F32, tag="sm")
                    nc.vector.tensor_add(tm[:], ps[:], mask_sb[:])
                    p4 = sm_pool.tile([128, 128], F32, tag="sm")
                    s4 = small_pool.tile([128, 1], F32, tag="s4")
                    nc.scalar.activation(
                        p4[:], tm[:], mybir.ActivationFunctionType.Exp,
                        scale=float(SCALE), accum_out=s4[:],
                    )
                    r4 = small_pool.tile([128, 1], F32, tag="r4")
                    nc.vector.reciprocal(r4[:], s4[:])
                    pn = smh_pool.tile([128, 128], F16, tag="smh")
                    nc.vector.tensor_scalar_mul(pn[:], p4[:], r4[:])
                    pt = smh_pool.tile([128, 128], F16, tag="smh")
                    nc.vector.transpose(pt[:], pn[:])
                    pt_sb.append(pt)

                # --- v' natural [tok-chunk 128, 1024 dout] and p@v' in
                # natural layout, interleaved one pack behind so the v'
                # evicts (vector/scalar alternating) are never on the PE
                # critical path.  p@v' psum is [128 qtok, 512 dout] at
                # full N=512; its evict adds the free-axis bias from a
                # replicated tile and the [128,1024] output of each pack
                # leaves as a single 256KB DMA.
                v_sb = [None] * QP

                def vproj(tch):
                    vt = v_pool.tile([128, D], F16, tag="v")
                    for hh in range(2):
                        ps = psA.tile([128, 512], F32, tag="psA")
                        for c in range(DC):
                            o = 512 * c + 128 * tch
                            nc.tensor.matmul(
                                ps[:],
                                lhsT=xt_sb[:, o:o + 128],
                                rhs=wf_sb[:, 1024 * c + 512 * hh:
                                          1024 * c + 512 * (hh + 1)],
                                start=(c == 0), stop=(c == DC - 1),
                            )
                        dst = vt[:, 512 * hh:512 * (hh + 1)]
                        if hh == 0:
                            nc.vector.tensor_copy(dst, ps[:])
                        else:
                            nc.scalar.copy(dst, ps[:])
                    v_sb[tch] = vt

                def pv(qp, b=b, pt_sb=pt_sb, v_sb=v_sb):
                    o = out_pool.tile([128, D], F16, tag="o")
                    r0 = 512 * b + 128 * qp
                    for hh in range(2):
                        ps = psB.tile([128, 512], F32, tag="psB")
                        nc.tensor.matmul(
                            ps[:],
                            lhsT=pt_sb[qp][:],
                            rhs=v_sb[qp][:, 512 * hh:512 * (hh + 1)],
                            start=True, stop=True,
                        )
                        dst = o[:, 512 * hh:512 * (hh + 1)]
                        src = bo_sb[:, 512 * hh:512 * (hh + 1)]
                        # evict halves on vector, each half DMA'd on its
                        # own queue as soon as it lands
                        nc.vector.tensor_add(dst, ps[:], src)
                        eng = nc.sync if hh == 0 else nc.gpsimd
                        eng.dma_start(
                            ot[r0:r0 + 128, 512 * hh:512 * (hh + 1)], dst)

                vproj(0)
                vproj(1)
                pv(0)
                vproj(2)
                pv(1)
                vproj(3)
                pending = [lambda pv=pv: pv(2), lambda pv=pv: pv(3)]

            for fn in pending:
                fn()

    nc.compile()
    return nc


def _make_mask():
    """One [128, 128] additive-mask tile shared by every q-pack: rows
    and columns are the pack's own 4 groups x 32 positions; the group-
    diagonal blocks carry the causal mask, everything else NEG
    (-> exp == 0 exactly)."""
    m = np.full((128, 128), NEG, dtype=np.float32)
    for i in range(4):
        for q in range(NB):
            m[32 * i + q, 32 * i:32 * i + q + 1] = 0.0
    return m


def run(x, Wqkv, bqkv, Wout, bout, trace=False):
    x = np.asarray(x, dtype=np.float32)
    Wqkv = np.asarray(Wqkv, dtype=np.float32)
    bqkv = np.asarray(bqkv, dtype=np.float32)
    Wout = np.asarray(Wout, dtype=np.float32)
    bout = np.asarray(bout, dtype=np.float32)

    # (B, S, D) -> (group, nb, D), group = b*BL + bl, token = g*NB + nb
    xg = x.reshape(B, NB, BL, D).transpose(0, 2, 1, 3).reshape(NGROUP, NB, D)
    Wq = Wqkv[:, :D]
    Wk = Wqkv[:, D:2 * D]
    Wv = Wqkv[:, 2 * D:3 * D]
    bq = bqkv[:D]
    bv = bqkv[2 * D:3 * D]
    # folds: scores = x (Wq Wk^T) x^T + (bq Wk^T) x^T  (bk cancels);
    # out = p (x (Wv Wout)) + (bv Wout + bout)  (softmax rows sum to 1)
    A = (Wq @ Wk.T).astype(np.float16)
    ba = np.ascontiguousarray(bq @ Wk.T).astype(np.float32)
    Wf = (Wv @ Wout).astype(np.float16)
    bo1 = (bout + bv @ Wout).astype(np.float32)
    bo = np.ascontiguousarray(np.broadcast_to(bo1, (128, D)))
    mask = _make_mask()

    # super-tile packings (layouts documented in _build_program)
    wa = np.ascontiguousarray(
        A.reshape(DC, 128, 2, 4, 128).transpose(1, 2, 0, 3, 4)
        .reshape(128, D * DC))
    wf = np.ascontiguousarray(
        Wf.reshape(DC, 128, D).transpose(1, 0, 2).reshape(128, D * DC))

    nc = _get_program()
    in_maps = []
    for i in range(N_CORES):
        xt_i = xg[GPC * i:GPC * (i + 1)].reshape(TOK, D).T
        xt_i = np.ascontiguousarray(
            xt_i.reshape(DC, 128, NBLK, BLK).transpose(2, 1, 0, 3)
            .reshape(NBLK * 128, DC * BLK)).astype(np.float16)
        in_maps.append({
            "xt": xt_i, "wa": wa, "wf": wf,
            "ba": ba, "bo": bo, "mask": mask,
        })
    res = run_bass_kernel_spmd(nc, in_maps, list(range(N_CORES)), trace=trace)

    outs = np.empty((NGROUP, NB, D), dtype=np.float32)
    for i in range(N_CORES):
        ot_i = res.results[i]["ot"].astype(np.float32)
        outs[GPC * i:GPC * (i + 1)] = ot_i.reshape(GPC, NB, D)
    out = (outs.reshape(B, BL, NB, D).transpose(0, 2, 1, 3)
           .reshape(B, S, D))
    return out, res


def kernel(x, Wqkv, bqkv, Wout, bout):
    out, _ = run(x, Wqkv, bqkv, Wout, bout, trace=False)
    return out
